# revision 38
# baseline (speedup 1.0000x reference)
"""Multi-head attention (B=2, S=2048, D=1024, H=16) on 8 TRN2 NeuronCores.

Sharding (Megatron-style, hardcoded):
  - batch b = core // 4  (2 groups of 4 cores)
  - head group g = core % 4 -> heads [4g, 4g+4), feature slice F = 256 rows
    of w_q/w_k/w_v (column-parallel) and 256 columns of w_out (row-parallel).
Each core computes a full [S, D] partial of the output for its batch
(summed over its 256 ctx features); the host sums the 4 partials per batch
and adds b_out.

On-core design (v7) — the ACT engine (exp) is the pacer: 128 exps of
[128,1024] x ~1.11us = 142us is the floor, so everything else is cargo
braided into the 8 ACT-paced score windows:
  - x tensors live in DRAM as [128, DT, S] so one 512-column chunk is a
    single DMA descriptor ([128, DT, 512]); the SP sequencer costs ~600ns
    per descriptor and was the v6 head bottleneck.
  - head (~22us): short warmup, k-proj c0 + q-proj c0 + half of k-proj c1
    as soon as their DMAs land; first exp fires ~22us (v5: 50us).
  - w0 = scores(0,0) + k-proj c1(rest),c2,c3 just ahead of their consuming
    kt + vt 0-3;  w1 = scores(1,0) + vt 4-15 + q-proj c1.
  - w2..w7 = scores(job) + ctx(job-2) + cargo (q2, q3, out-proj tt0-7).
  - tail = ctx(0,3); norm(0,3); ctx(1,3) (PSUM banks borrowed from the
    now-idle scores pool) overlapping norm(0,3) DVE chain; tt8-11
    overlapping norm(1,3); tt12-15. MMs stay back-to-back for HAM.
  - per-job p is stored as two [128,8,1024] halves (bufs=6) so the
    rotation frees at half-window granularity (v6 stalled ~1us per
    window start on whole-p WAR).
Per-op idioms unchanged from v5 (all HW-verified): paired K=64 score
matmuls into one [128,1024] PSUM tile -> single exp for 2 heads x 512
queries; v^T by x-stationary projection; ones-column softmax denominator
(M=65 ctx stationary); h' ctx normalized into a bp-0 temp and DMA-shifted
to partitions 64-127 (DVE lanes cannot cross partitions; the l-row is
tensor_copy'd off partition 64 before the custom-DVE reciprocal).
Softmax skips the max-subtraction: scores ~ N(0,1), exp never overflows.
"""

import os

import numpy as np

import concourse.bass as bass
import concourse.tile as tile
from concourse import bacc, mybir
from concourse.bass_utils import run_bass_kernel_spmd

B, S, D, H, DK = 2, 2048, 1024, 16, 64
N_CORES = 8
GROUPS = 4              # head-groups (cores per batch)
HL = H // GROUPS        # heads per core = 4
F = HL * DK             # feature slice per core = 256
FT = F // 128           # f-tiles per core = 2
DT = D // 128           # d-tiles (contraction) = 8
KT = S // 128           # 128-wide key tiles = 16
WQ = S // 512           # 512-wide query chunks = 4
KC = S // 512           # 512-wide key chunks for k-proj = 4

F32 = mybir.dt.float32
BF16 = mybir.dt.bfloat16
AFT = mybir.ActivationFunctionType

_CACHE = {}
LAST_RESULTS = None  # BassKernelResults of the most recent run (for test.py)


def _build():
    nc = bacc.Bacc("TRN2", target_bir_lowering=False, debug=False,
                   num_devices=N_CORES)

    xq = nc.declare_dram_parameter("xq_t", [KC, 128, DT, 512], BF16,
                                   isOutput=False)
    xk = nc.declare_dram_parameter("xk_t", [KC, 128, DT, 512], BF16,
                                   isOutput=False)
    xv = nc.declare_dram_parameter("xv_t", [KC, 128, DT, 512], BF16,
                                   isOutput=False)
    wq = nc.declare_dram_parameter("wq_t", [128, DT, F], BF16, isOutput=False)
    wk = nc.declare_dram_parameter("wk_t", [128, DT, F], BF16, isOutput=False)
    wv = nc.declare_dram_parameter("wv_t2", [DT, 128, F], BF16, isOutput=False)
    bq = nc.declare_dram_parameter("bq", [128, FT], F32, isOutput=False)
    bk = nc.declare_dram_parameter("bk", [128, FT], F32, isOutput=False)
    bv = nc.declare_dram_parameter("bv_row", [1, F], F32, isOutput=False)
    wo = nc.declare_dram_parameter("wo_t", [128, FT, D], BF16, isOutput=False)
    out = nc.declare_dram_parameter("out_p", [S, D], BF16, isOutput=True)

    with tile.TileContext(nc) as tc:
        with (
            tc.tile_pool(name="const", bufs=1) as const,
            tc.tile_pool(name="acts", bufs=1) as acts,
            tc.tile_pool(name="ppool", bufs=6) as ppool,
            tc.tile_pool(name="xkpool", bufs=1) as xkpool,
            tc.tile_pool(name="xqpool", bufs=1) as xqpool,
            tc.tile_pool(name="wpool", bufs=1) as wpool,
            tc.tile_pool(name="small", bufs=1) as small,
            tc.tile_pool(name="opool", bufs=2) as opool,
            tc.tile_pool(name="psS", bufs=2, space="PSUM") as psS,
            tc.tile_pool(name="psC", bufs=2, space="PSUM") as psC,
            tc.tile_pool(name="psA", bufs=2, space="PSUM") as psA,
        ):
            scratch = const.tile([128, 512], BF16, tag="scratch")
            nc.gpsimd.memset(scratch[:], 0.0)

            # persistent activations
            q_sb = acts.tile([128, FT, S], BF16, tag="q")
            k_sb = acts.tile([128, FT, S], BF16, tag="k")
            vt_sb = acts.tile([128, HL, KT, 65], BF16, tag="vt")
            ctx_sb = acts.tile([128, FT, S], BF16, tag="ctx")
            nc.vector.memset(vt_sb[:, :, :, 64:65], 1.0)

            def xchunk(pool, pfx, c, src=None, eng=None):
                t = pool.tile([128, DT, 512], BF16, tag="x" + pfx,
                              bufs=(2 if pfx == "q" else KC),
                              name=f"x{pfx}{c}")
                if src is not None:
                    (eng or nc.sync).dma_start(out=t[:], in_=src[c])
                return t

            # ---- DMA part 1: what the head needs.  One descriptor rides
            # ONE of the 16 DMA engines (~190 B/ns), so the
            # latency-critical head transfers are split into partition-
            # range sub-descriptors that stream in parallel. ----
            def split_dma(t, src, n):
                step = 128 // n
                for r in range(n):
                    ps = slice(r * step, (r + 1) * step)
                    nc.sync.dma_start(out=t[ps], in_=src[ps])

            w_sb = {}
            xk_t = [None] * KC
            xq_t = [None] * WQ
            xk_t[0] = xchunk(xkpool, "k", 0)
            split_dma(xk_t[0], xk[0], 4)
            w_sb["k"] = wpool.tile([128, DT, F], BF16, tag="wk", name="wk_sb")
            split_dma(w_sb["k"], wk, 2)
            xq_t[0] = xchunk(xqpool, "q", 0)
            split_dma(xq_t[0], xq[0], 4)
            w_sb["q"] = wpool.tile([128, DT, F], BF16, tag="wq", name="wq_sb")
            split_dma(w_sb["q"], wq, 2)
            xk_t[1] = xchunk(xkpool, "k", 1)
            split_dma(xk_t[1], xk[1], 2)
            b_sb = {}
            for name, bp in (("k", bk), ("q", bq)):
                b_sb[name] = const.tile([128, FT], F32, tag=f"b{name}",
                                        name=f"b{name}_sb")
                nc.sync.dma_start(out=b_sb[name][:], in_=bp[:])
            bv_row = const.tile([1, F], F32, tag="bvrow")
            nc.sync.dma_start(out=bv_row[:], in_=bv[:])
            bv_b = const.tile([128, F], F32, tag="bvb")
            nc.gpsimd.partition_broadcast(bv_b[:], bv_row[:])

            # ---- head compute: warmup + k-proj c0 + q-proj c0 + kc1 ----
            warm = psA.tile([128, 512], F32, tag="ps2", name="warm")
            for i in range(20):
                nc.tensor.matmul(warm[:], scratch[:, 0:128], scratch[:],
                                 start=True, stop=True)

            def kproj_quad(c, qd, tiles):
                """Emit dt = 2*qd, 2*qd+1 (x fi 0,1) of k-proj chunk c into
                tiles [fi0, fi1]; at qd==3 add bias into k_sb."""
                ks = slice(c * 512, (c + 1) * 512)
                for dt in (2 * qd, 2 * qd + 1):
                    for fi in range(FT):
                        nc.tensor.matmul(
                            tiles[fi][:],
                            w_sb["k"][:, dt, fi * 128:(fi + 1) * 128],
                            xk_t[c][:, dt, :],
                            start=(dt == 0), stop=(dt == DT - 1),
                        )
                if qd == 3:
                    for fi in range(FT):
                        nc.vector.tensor_scalar_add(
                            out=k_sb[:, fi, ks], in0=tiles[fi][:],
                            scalar1=b_sb["k"][:, fi:fi + 1],
                        )

            def kproj_alloc(c):
                return [psA.tile([128, 512], F32, tag="ps2",
                                 name=f"kb{c}_{fi}") for fi in range(FT)]

            def qproj_mm(c, j, tiles):
                """Emit MM j (0..15) of q-proj chunk c: dt=j//2, fi=j%2."""
                ws = slice(c * 512, (c + 1) * 512)
                dt, fi = j // 2, j % 2
                nc.tensor.matmul(
                    tiles[fi][:],
                    w_sb["q"][:, dt, fi * 128:(fi + 1) * 128],
                    xq_t[c][:, dt, :], start=(dt == 0), stop=(dt == DT - 1),
                )
                if j == 15:
                    for fi in range(FT):
                        nc.vector.tensor_scalar_add(
                            out=q_sb[:, fi, ws], in0=tiles[fi][:],
                            scalar1=b_sb["q"][:, fi:fi + 1],
                        )

            def qproj_alloc(c):
                return [psA.tile([128, 512], F32, tag="ps2",
                                 name=f"qb{c}_{fi}") for fi in range(FT)]

            kp_tiles, qp_tiles = {}, {}

            kp_tiles[0] = kproj_alloc(0)
            for qd in range(4):
                kproj_quad(0, qd, kp_tiles[0])
            qp_tiles[0] = qproj_alloc(0)
            for j in range(16):
                qproj_mm(0, j, qp_tiles[0])
            # ---- DMA part 2 ----
            for c in range(2, KC):
                xk_t[c] = xchunk(xkpool, "k", c, xk)
            wv_sb = wpool.tile([128, DT, F], BF16, tag="wv")
            for dt in range(DT):
                nc.sync.dma_start(out=wv_sb[:, dt, :], in_=wv[dt])
            # xv chunk tiles reuse the xk slots (kproj chunk c's reads are
            # long done before xv chunk c lands)
            xv_t = [None] * KC
            xv_t[0] = xchunk(xkpool, "k", 0, xv)
            xq_t[1] = xchunk(xqpool, "q", 1, xq)
            xv_t[1] = xchunk(xkpool, "k", 1, xv)
            # head gets a 2-quad head start on k-proj c1
            kp_tiles[1] = kproj_alloc(1)
            kproj_quad(1, 0, kp_tiles[1])
            kproj_quad(1, 1, kp_tiles[1])

            # ---- braid machinery ----
            def vt_kt(kt):
                vb = psA.tile([128, 512], F32, tag="ps2", name=f"vtb{kt}")
                cc, ko = kt // 4, (kt % 4) * 128
                for dt in range(DT):
                    nc.tensor.matmul(
                        vb[:, 0:F], xv_t[cc][:, dt, ko:ko + 128],
                        wv_sb[:, dt, :],
                        start=(dt == 0), stop=(dt == DT - 1),
                    )
                nc.vector.tensor_add(
                    vt_sb[:, :, kt, 0:64], vb[:, 0:F], bv_b[:])

            o_tiles = {}

            def half_tt(tt, j, pool=None, dma_eng=None, copy_scalar=False):
                """Half out-proj for row-tile tt: out cols [512j, 512j+512)."""
                pool = pool if pool is not None else psA
                dma_eng = dma_eng if dma_eng is not None else nc.sync
                ts = slice(tt * 128, (tt + 1) * 128)
                js = slice(j * 512, (j + 1) * 512)
                if j == 0:
                    o_tiles[tt] = opool.tile([128, D], BF16, tag="o",
                                             name=f"o{tt}")
                o_t = o_tiles[tt]
                ob = pool.tile([128, 512], F32,
                               tag=("ps2" if pool is psA else "s"),
                               name=f"ob{tt}_{j}")
                for fi in range(FT):
                    nc.tensor.matmul(
                        ob[:], ctx_sb[:, fi, ts], wo_sb[:, fi, js],
                        start=(fi == 0), stop=(fi == FT - 1),
                    )
                if copy_scalar:
                    nc.scalar.copy(o_t[:, js], ob[:])
                else:
                    nc.vector.tensor_copy(o_t[:, js], ob[:])
                if j == 1:
                    dma_eng.dma_start(out=out[ts, :], in_=o_t[:])

            p_tiles = {}

            def scores_exp(hp, wq_i, cargo=None):
                ws = slice(wq_i * 512, (wq_i + 1) * 512)
                p_pair = (
                    ppool.tile([128, KT // 2, 1024], BF16, tag="p",
                               name=f"p{hp}_{wq_i}a"),
                    ppool.tile([128, KT // 2, 1024], BF16, tag="p",
                               name=f"p{hp}_{wq_i}b"),
                )
                p_tiles[(hp, wq_i)] = p_pair
                for kt in range(KT):
                    ks = slice(kt * 128, (kt + 1) * 128)
                    s_t = psS.tile([128, 1024], F32, tag="s",
                                   name=f"s{hp}{wq_i}_{kt}")
                    nc.tensor.matmul(
                        s_t[:, 0:512], k_sb[0:64, hp, ks],
                        q_sb[0:64, hp, ws], start=True, stop=True,
                    )
                    nc.tensor.matmul(
                        s_t[:, 512:1024], k_sb[64:128, hp, ks],
                        q_sb[64:128, hp, ws], start=True, stop=True,
                    )
                    nc.scalar.activation(
                        p_pair[kt // 8][:, kt % 8, :], s_t[:], AFT.Exp)
                    if cargo is not None:
                        cargo(kt)

            def ctx_mms(hp, wq_i, c_h0, c_h1, kt):
                h0, h1 = 2 * hp, 2 * hp + 1
                p_half = p_tiles[(hp, wq_i)][kt // 8]
                nc.tensor.matmul(
                    c_h0[:], vt_sb[:, h0, kt, :], p_half[:, kt % 8, 0:512],
                    start=(kt == 0), stop=(kt == KT - 1),
                )
                nc.tensor.matmul(
                    c_h1[:], vt_sb[:, h1, kt, :],
                    p_half[:, kt % 8, 512:1024],
                    start=(kt == 0), stop=(kt == KT - 1),
                )

            def ctx_alloc(hp, wq_i, pool=None, tag="c"):
                pool = pool if pool is not None else psC
                c_h0 = pool.tile([65, 512], F32, tag=tag,
                                 name=f"c{hp}{wq_i}a")
                c_h1 = pool.tile([65, 512], F32, tag=tag,
                                 name=f"c{hp}{wq_i}b")
                return c_h0, c_h1

            def ctx_norm(hp, wq_i, mms_done, dma_eng=None):
                dma_eng = dma_eng if dma_eng is not None else nc.sync
                h0, h1 = 2 * hp, 2 * hp + 1
                ws = slice(wq_i * 512, (wq_i + 1) * 512)
                c_h0, c_h1 = mms_done
                p_tiles.pop((hp, wq_i))
                # normalize h0 -> ctx_sb[0:64]; h1 -> tmp + DMA shift.
                # (the l-row must be tensor_copy'd off partition 64 first:
                # a custom-DVE op straight from PSUM@p64 to SBUF@p0
                # returns garbage on HW)
                lrow0 = small.tile([1, 512], F32, tag="lr",
                                   name=f"lr0_{hp}{wq_i}")
                nc.vector.tensor_copy(lrow0[:], c_h0[64:65, :])
                linv0 = small.tile([1, 512], F32, tag="linv",
                                   name=f"l0_{hp}{wq_i}")
                nc.vector.reciprocal_approx_fast(linv0[:], lrow0[:])
                lb0 = small.tile([64, 512], F32, tag="lb",
                                 name=f"lb0_{hp}{wq_i}")
                nc.gpsimd.partition_broadcast(lb0[:], linv0[:])
                nc.vector.tensor_mul(
                    ctx_sb[0:64, hp, ws], c_h0[0:64, :], lb0[:])

                lrow1 = small.tile([1, 512], F32, tag="lr",
                                   name=f"lr1_{hp}{wq_i}")
                nc.vector.tensor_copy(lrow1[:], c_h1[64:65, :])
                linv1 = small.tile([1, 512], F32, tag="linv",
                                   name=f"l1_{hp}{wq_i}")
                nc.vector.reciprocal_approx_fast(linv1[:], lrow1[:])
                lb1 = small.tile([64, 512], F32, tag="lb",
                                 name=f"lb1_{hp}{wq_i}")
                nc.gpsimd.partition_broadcast(lb1[:], linv1[:])
                tmp1 = small.tile([64, 512], BF16, tag="tmp",
                                  name=f"t1_{hp}{wq_i}")
                nc.vector.tensor_mul(tmp1[:], c_h1[0:64, :], lb1[:])
                dma_eng.dma_start(out=ctx_sb[64:128, hp, ws], in_=tmp1[:])

            # cargo closures per window
            def w0_cargo(kt):
                if kt < 2:
                    kproj_quad(1, 2 + kt, kp_tiles[1])
                elif kt < 6:
                    if kt == 2:
                        kp_tiles[2] = kproj_alloc(2)
                    kproj_quad(2, kt - 2, kp_tiles[2])
                elif kt < 10:
                    if kt == 6:
                        kp_tiles[3] = kproj_alloc(3)
                    kproj_quad(3, kt - 6, kp_tiles[3])
                elif kt < 14:
                    vt_kt(kt - 10)

            def w1_cargo(kt):
                if kt < 12:
                    vt_kt(4 + kt)
                else:
                    if kt == 12:
                        qp_tiles[1] = qproj_alloc(1)
                    for j in range(4 * (kt - 12), 4 * (kt - 12) + 4):
                        qproj_mm(1, j, qp_tiles[1])

            # ctx pairs slide to slots 4-15 (doubles early, at 5/7/9/11)
            # so the first pair never waits on the previous job's norm
            # chain (psC bank WAR); the other cargo rides slots 0-3
            # (q-proj) or 12-15 (out-proj, whose DVE copies would collide
            # with the norm chain if run at the window start).
            CTX_SLOTS = {4: (0,), 5: (1, 2), 6: (3,), 7: (4, 5), 8: (6,),
                         9: (7, 8), 10: (9,), 11: (10, 11), 12: (12,),
                         13: (13,), 14: (14,), 15: (15,)}

            def ctx_cargo(hp, wq_i, extra=None, extra_slots=(0, 1, 2, 3)):
                c = ctx_alloc(hp, wq_i)

                def cargo(kt):
                    for ckt in CTX_SLOTS.get(kt, ()):
                        ctx_mms(hp, wq_i, c[0], c[1], ckt)
                    if extra is not None and kt in extra_slots:
                        extra(extra_slots.index(kt))
                return c, cargo

            def qproj_cargo(c):
                """q-proj chunk c as 4-MM quads."""
                def cargo(s):
                    if s == 0:
                        qp_tiles[c] = qproj_alloc(c)
                    for j in range(4 * s, 4 * s + 4):
                        qproj_mm(c, j, qp_tiles[c])
                return cargo

            def ttq_cargo(tt_base):
                """4 half-tts (= 2 full tt)."""
                def cargo(s):
                    half_tt(tt_base + s // 2, s % 2)
                return cargo

            # ---- the 8 braided windows ----
            scores_exp(0, 0, cargo=w0_cargo)                       # w0
            # DMA part 3
            xv_t[2] = xchunk(xkpool, "k", 2, xv)
            xv_t[3] = xchunk(xkpool, "k", 3, xv)
            wo_sb = wpool.tile([128, FT, D], BF16, tag="wo")
            nc.sync.dma_start(out=wo_sb[:], in_=wo[:])
            xq_t[2] = xchunk(xqpool, "q", 2, xq)
            scores_exp(1, 0, cargo=w1_cargo)                       # w1
            xq_t[3] = xchunk(xqpool, "q", 3, xq)

            c00, h00 = ctx_cargo(0, 0, extra=qproj_cargo(2))
            scores_exp(0, 1, cargo=h00)                            # w2
            ctx_norm(0, 0, mms_done=c00)
            c10, h10 = ctx_cargo(1, 0, extra=qproj_cargo(3))
            scores_exp(1, 1, cargo=h10)                            # w3
            ctx_norm(1, 0, mms_done=c10)
            TT_SLOTS = (12, 13, 14, 15)
            c01, h01 = ctx_cargo(0, 1, extra=ttq_cargo(0),
                                 extra_slots=TT_SLOTS)
            scores_exp(0, 2, cargo=h01)                            # w4
            ctx_norm(0, 1, mms_done=c01)
            c11, h11 = ctx_cargo(1, 1, extra=ttq_cargo(2),
                                 extra_slots=TT_SLOTS)
            scores_exp(1, 2, cargo=h11)                            # w5
            ctx_norm(1, 1, mms_done=c11)
            c02, h02 = ctx_cargo(0, 2, extra=ttq_cargo(4),
                                 extra_slots=TT_SLOTS)
            scores_exp(0, 3, cargo=h02)                            # w6
            ctx_norm(0, 2, mms_done=c02)
            c12, h12 = ctx_cargo(1, 2, extra=ttq_cargo(6),
                                 extra_slots=TT_SLOTS)
            scores_exp(1, 3, cargo=h12)                            # w7
            ctx_norm(1, 2, mms_done=c12)

            # ---- dense tail ----
            # ctx(0,3) borrows the scores PSUM banks (scores are done) so
            # it does not wait on norm(1,2)'s reads of the psC banks;
            # ctx(1,3) takes psC once norm(1,2) drains (hidden under
            # ctx(0,3)'s 7us of MMs).  Shift DMAs ride the idle ACT
            # engine's queue; tt out-proj MMs keep the PE hot under the
            # norm chains.
            c03 = ctx_alloc(0, 3, pool=psS, tag="s")
            for kt in range(KT):
                ctx_mms(0, 3, c03[0], c03[1], kt)
            ctx_norm(0, 3, mms_done=c03, dma_eng=nc.scalar)
            c13 = ctx_alloc(1, 3)
            for kt in range(KT):
                ctx_mms(1, 3, c13[0], c13[1], kt)
            half_tt(8, 0, pool=psS, copy_scalar=True)
            half_tt(8, 1)
            half_tt(9, 0, pool=psS, copy_scalar=True)
            half_tt(9, 1)
            ctx_norm(1, 3, mms_done=c13, dma_eng=nc.scalar)
            for tt in (10, 11, 12, 13, 14, 15):
                half_tt(tt, 0, pool=psS, copy_scalar=True)
                half_tt(tt, 1)

    nc.compile()
    return nc


def get_program():
    if "nc" not in _CACHE:
        _CACHE["nc"] = _build()
    return _CACHE["nc"]


def _bf(a):
    import ml_dtypes
    return a.astype(ml_dtypes.bfloat16)


def prep_in_maps(query_tensor, key_tensor, value_tensor, w_q, b_q, w_k, b_k,
                 w_v, b_v, w_out, b_out):
    """Per-core input dicts. Core c: batch c//4, feature rows [256*(c%4), ...)."""
    f32 = np.float32
    scale = f32(1.0 / np.sqrt(DK))

    def xt(x, b):  # [S, D] -> [KC, 128, DT, 512] (chunk-major, contiguous)
        return _bf(np.ascontiguousarray(
            np.asarray(x[b], f32).reshape(KC, 512, DT, 128)
            .transpose(0, 3, 2, 1)))

    xs = {"xq_t": [xt(query_tensor, b) for b in range(B)],
          "xk_t": [xt(key_tensor, b) for b in range(B)],
          "xv_t": [xt(value_tensor, b) for b in range(B)]}

    def wt(w, g, s=f32(1.0)):  # rows [256g, 256g+256) of w -> [128, DT, F]
        sl = np.asarray(w[256 * g:256 * (g + 1), :], f32) * s  # [F, D]
        return _bf(np.ascontiguousarray(
            sl.T.reshape(DT, 128, F).transpose(1, 0, 2)))

    def wvt(w, g):  # [DT, 128, F] (d-major, untransposed)
        sl = np.asarray(w[256 * g:256 * (g + 1), :], f32)  # [F, D]
        return _bf(np.ascontiguousarray(sl.T.reshape(DT, 128, F)))

    def bt(b_, g, s=f32(1.0)):  # [128, FT]
        sl = np.asarray(b_[256 * g:256 * (g + 1)], f32) * s
        return np.ascontiguousarray(sl.reshape(FT, 128).T)

    def wot(w, g):  # cols [256g, 256g+256) of w_out -> [128, FT, D]
        sl = np.asarray(w[:, 256 * g:256 * (g + 1)], f32)  # [D, F]
        return _bf(np.ascontiguousarray(
            sl.T.reshape(FT, 128, D).transpose(1, 0, 2)))

    in_maps = []
    for c in range(N_CORES):
        b, g = divmod(c, GROUPS)
        in_maps.append({
            "xq_t": xs["xq_t"][b], "xk_t": xs["xk_t"][b], "xv_t": xs["xv_t"][b],
            "wq_t": wt(w_q, g, scale), "wk_t": wt(w_k, g),
            "wv_t2": wvt(w_v, g),
            "bq": bt(b_q, g, scale), "bk": bt(b_k, g),
            "bv_row": np.ascontiguousarray(
                np.asarray(b_v[256 * g:256 * (g + 1)], f32).reshape(1, F)),
            "wo_t": wot(w_out, g),
        })
    return in_maps


def kernel(query_tensor, key_tensor, value_tensor, w_q, b_q, w_k, b_k,
           w_v, b_v, w_out, b_out):
    global LAST_RESULTS
    nc = get_program()
    in_maps = prep_in_maps(query_tensor, key_tensor, value_tensor, w_q, b_q,
                           w_k, b_k, w_v, b_v, w_out, b_out)
    res = run_bass_kernel_spmd(nc, in_maps, list(range(N_CORES)),
                               tmpdir=os.environ.get("BASS_TMPDIR"))
    LAST_RESULTS = res
    b_out = np.asarray(b_out, np.float32)
    out = np.empty((B, S, D), np.float32)
    for b in range(B):
        acc = res.results[4 * b]["out_p"].astype(np.float32)
        for g in range(1, GROUPS):
            acc = acc + res.results[4 * b + g]["out_p"]
        out[b] = acc + b_out
    return out


# revision 40
# speedup vs baseline: 1.0508x; 1.0508x over previous
"""Multi-head attention (B=2, S=2048, D=1024, H=16) on 8 TRN2 NeuronCores.

Sharding (Megatron-style, hardcoded):
  - batch b = core // 4  (2 groups of 4 cores)
  - head group g = core % 4 -> heads [4g, 4g+4), feature slice F = 256 rows
    of w_q/w_k/w_v (column-parallel) and 256 columns of w_out (row-parallel).
Each core computes a full [S, D] partial of the output for its batch
(summed over its 256 ctx features); the host sums the 4 partials per batch
and adds b_out.

On-core design (v7) — the ACT engine (exp) is the pacer: 128 exps of
[128,1024] x ~1.11us = 142us is the floor, so everything else is cargo
braided into the 8 ACT-paced score windows:
  - x tensors live in DRAM as [128, DT, S] so one 512-column chunk is a
    single DMA descriptor ([128, DT, 512]); the SP sequencer costs ~600ns
    per descriptor and was the v6 head bottleneck.
  - head (~22us): short warmup, k-proj c0 + q-proj c0 + half of k-proj c1
    as soon as their DMAs land; first exp fires ~22us (v5: 50us).
  - w0 = scores(0,0) + k-proj c1(rest),c2,c3 just ahead of their consuming
    kt + vt 0-3;  w1 = scores(1,0) + vt 4-15 + q-proj c1.
  - w2..w7 = scores(job) + ctx(job-2) + cargo (q2, q3, out-proj tt0-7).
  - tail = ctx(0,3); norm(0,3); ctx(1,3) (PSUM banks borrowed from the
    now-idle scores pool) overlapping norm(0,3) DVE chain; tt8-11
    overlapping norm(1,3); tt12-15. MMs stay back-to-back for HAM.
  - per-job p is stored as two [128,8,1024] halves (bufs=6) so the
    rotation frees at half-window granularity (v6 stalled ~1us per
    window start on whole-p WAR).
Per-op idioms unchanged from v5 (all HW-verified): paired K=64 score
matmuls into one [128,1024] PSUM tile -> single exp for 2 heads x 512
queries; v^T by x-stationary projection; ones-column softmax denominator
(M=65 ctx stationary); h' ctx normalized into a bp-0 temp and DMA-shifted
to partitions 64-127 (DVE lanes cannot cross partitions; the l-row is
tensor_copy'd off partition 64 before the custom-DVE reciprocal).
Softmax skips the max-subtraction: scores ~ N(0,1), exp never overflows.
"""

import os

import numpy as np

import concourse.bass as bass
import concourse.tile as tile
from concourse import bacc, mybir
from concourse.bass_utils import run_bass_kernel_spmd

B, S, D, H, DK = 2, 2048, 1024, 16, 64
N_CORES = 8
GROUPS = 4              # head-groups (cores per batch)
HL = H // GROUPS        # heads per core = 4
F = HL * DK             # feature slice per core = 256
FT = F // 128           # f-tiles per core = 2
DT = D // 128           # d-tiles (contraction) = 8
KT = S // 128           # 128-wide key tiles = 16
WQ = S // 512           # 512-wide query chunks = 4
KC = S // 512           # 512-wide key chunks for k-proj = 4

F32 = mybir.dt.float32
BF16 = mybir.dt.bfloat16
AFT = mybir.ActivationFunctionType

_CACHE = {}
LAST_RESULTS = None  # BassKernelResults of the most recent run (for test.py)


def _build():
    nc = bacc.Bacc("TRN2", target_bir_lowering=False, debug=False,
                   num_devices=N_CORES)

    xq = nc.declare_dram_parameter("xq_t", [KC, 128, DT, 512], BF16,
                                   isOutput=False)
    xk = nc.declare_dram_parameter("xk_t", [KC, 128, DT, 512], BF16,
                                   isOutput=False)
    xv = nc.declare_dram_parameter("xv_t", [KC, 128, DT, 512], BF16,
                                   isOutput=False)
    wq = nc.declare_dram_parameter("wq_t", [128, DT, F], BF16, isOutput=False)
    wk = nc.declare_dram_parameter("wk_t", [128, DT, F], BF16, isOutput=False)
    wv = nc.declare_dram_parameter("wv_t2", [DT, 128, F], BF16, isOutput=False)
    bq = nc.declare_dram_parameter("bq", [128, FT], F32, isOutput=False)
    bk = nc.declare_dram_parameter("bk", [128, FT], F32, isOutput=False)
    bv = nc.declare_dram_parameter("bv_row", [1, F], F32, isOutput=False)
    wo = nc.declare_dram_parameter("wo_t", [128, FT, D], BF16, isOutput=False)
    out = nc.declare_dram_parameter("out_p", [S, D], BF16, isOutput=True)

    with tile.TileContext(nc) as tc:
        with (
            tc.tile_pool(name="const", bufs=1) as const,
            tc.tile_pool(name="acts", bufs=1) as acts,
            tc.tile_pool(name="ppool", bufs=6) as ppool,
            tc.tile_pool(name="xkpool", bufs=1) as xkpool,
            tc.tile_pool(name="xqpool", bufs=1) as xqpool,
            tc.tile_pool(name="wpool", bufs=1) as wpool,
            tc.tile_pool(name="small", bufs=1) as small,
            tc.tile_pool(name="opool", bufs=2) as opool,
            tc.tile_pool(name="psS", bufs=2, space="PSUM") as psS,
            tc.tile_pool(name="psC", bufs=2, space="PSUM") as psC,
            tc.tile_pool(name="psA", bufs=2, space="PSUM") as psA,
        ):
            scratch = const.tile([128, 512], BF16, tag="scratch")
            nc.gpsimd.memset(scratch[:], 0.0)

            # persistent activations
            q_sb = acts.tile([128, FT, S], BF16, tag="q")
            k_sb = acts.tile([128, FT, S], BF16, tag="k")
            vt_sb = acts.tile([128, HL, KT, 65], BF16, tag="vt")
            ctx_sb = acts.tile([128, FT, S], BF16, tag="ctx")
            nc.vector.memset(vt_sb[:, :, :, 64:65], 1.0)

            def xchunk(pool, pfx, c, src=None, eng=None):
                t = pool.tile([128, DT, 512], BF16, tag="x" + pfx,
                              bufs=(2 if pfx == "q" else KC),
                              name=f"x{pfx}{c}")
                if src is not None:
                    (eng or nc.sync).dma_start(out=t[:], in_=src[c])
                return t

            # ---- DMA part 1: what the head needs (the early aggregate
            # DMA rate ~225 B/ns is the physical bound; fewer, bigger
            # descriptors win because SP descriptor-gen is 600ns each) ----
            w_sb = {}
            xk_t = [None] * KC
            xq_t = [None] * WQ
            w_sb["k"] = wpool.tile([128, DT, F], BF16, tag="wk", name="wk_sb")
            nc.sync.dma_start(out=w_sb["k"][:], in_=wk[:])
            xk_t[0] = xchunk(xkpool, "k", 0, xk)
            w_sb["q"] = wpool.tile([128, DT, F], BF16, tag="wq", name="wq_sb")
            nc.sync.dma_start(out=w_sb["q"][:], in_=wq[:])
            xq_t[0] = xchunk(xqpool, "q", 0, xq)
            b_sb = {}
            for name, bp in (("k", bk), ("q", bq)):
                b_sb[name] = const.tile([128, FT], F32, tag=f"b{name}",
                                        name=f"b{name}_sb")
                nc.sync.dma_start(out=b_sb[name][:], in_=bp[:])
            bv_row = const.tile([1, F], F32, tag="bvrow")
            nc.sync.dma_start(out=bv_row[:], in_=bv[:])
            bv_b = const.tile([128, F], F32, tag="bvb")
            nc.gpsimd.partition_broadcast(bv_b[:], bv_row[:])

            # ---- head compute: warmup + k-proj c0 + q-proj c0 + kc1 ----
            warm = psA.tile([128, 512], F32, tag="ps2", name="warm")
            for i in range(20):
                nc.tensor.matmul(warm[:], scratch[:, 0:128], scratch[:],
                                 start=True, stop=True)

            def kproj_quad(c, qd, tiles):
                """Emit dt = 2*qd, 2*qd+1 (x fi 0,1) of k-proj chunk c into
                tiles [fi0, fi1]; at qd==3 add bias into k_sb."""
                ks = slice(c * 512, (c + 1) * 512)
                for dt in (2 * qd, 2 * qd + 1):
                    for fi in range(FT):
                        nc.tensor.matmul(
                            tiles[fi][:],
                            w_sb["k"][:, dt, fi * 128:(fi + 1) * 128],
                            xk_t[c][:, dt, :],
                            start=(dt == 0), stop=(dt == DT - 1),
                        )
                if qd == 3:
                    for fi in range(FT):
                        nc.vector.tensor_scalar_add(
                            out=k_sb[:, fi, ks], in0=tiles[fi][:],
                            scalar1=b_sb["k"][:, fi:fi + 1],
                        )

            def kproj_alloc(c):
                return [psA.tile([128, 512], F32, tag="ps2",
                                 name=f"kb{c}_{fi}") for fi in range(FT)]

            def qproj_mm(c, j, tiles):
                """Emit MM j (0..15) of q-proj chunk c: dt=j//2, fi=j%2."""
                ws = slice(c * 512, (c + 1) * 512)
                dt, fi = j // 2, j % 2
                nc.tensor.matmul(
                    tiles[fi][:],
                    w_sb["q"][:, dt, fi * 128:(fi + 1) * 128],
                    xq_t[c][:, dt, :], start=(dt == 0), stop=(dt == DT - 1),
                )
                if j == 15:
                    for fi in range(FT):
                        nc.vector.tensor_scalar_add(
                            out=q_sb[:, fi, ws], in0=tiles[fi][:],
                            scalar1=b_sb["q"][:, fi:fi + 1],
                        )

            def qproj_alloc(c):
                return [psA.tile([128, 512], F32, tag="ps2",
                                 name=f"qb{c}_{fi}") for fi in range(FT)]

            kp_tiles, qp_tiles = {}, {}

            kp_tiles[0] = kproj_alloc(0)
            for qd in range(4):
                kproj_quad(0, qd, kp_tiles[0])
            qp_tiles[0] = qproj_alloc(0)
            for j in range(16):
                qproj_mm(0, j, qp_tiles[0])
            # ---- DMA part 2 ----
            for c in range(1, KC):
                xk_t[c] = xchunk(xkpool, "k", c, xk)
            wv_sb = wpool.tile([128, DT, F], BF16, tag="wv")
            for dt in range(DT):
                nc.sync.dma_start(out=wv_sb[:, dt, :], in_=wv[dt])
            # xv chunk tiles reuse the xk slots (kproj chunk c's reads are
            # long done before xv chunk c lands)
            xv_t = [None] * KC
            xv_t[0] = xchunk(xkpool, "k", 0, xv)
            xq_t[1] = xchunk(xqpool, "q", 1, xq)
            xv_t[1] = xchunk(xkpool, "k", 1, xv)
            # head gets a 2-quad head start on k-proj c1
            kp_tiles[1] = kproj_alloc(1)
            kproj_quad(1, 0, kp_tiles[1])
            kproj_quad(1, 1, kp_tiles[1])

            # ---- braid machinery ----
            def vt_kt(kt):
                vb = psA.tile([128, 512], F32, tag="ps2", name=f"vtb{kt}")
                cc, ko = kt // 4, (kt % 4) * 128
                for dt in range(DT):
                    nc.tensor.matmul(
                        vb[:, 0:F], xv_t[cc][:, dt, ko:ko + 128],
                        wv_sb[:, dt, :],
                        start=(dt == 0), stop=(dt == DT - 1),
                    )
                nc.vector.tensor_add(
                    vt_sb[:, :, kt, 0:64], vb[:, 0:F], bv_b[:])

            o_tiles = {}

            def half_tt(tt, j, pool=None, dma_eng=None, copy_scalar=False):
                """Half out-proj for row-tile tt: out cols [512j, 512j+512)."""
                pool = pool if pool is not None else psA
                dma_eng = dma_eng if dma_eng is not None else nc.sync
                ts = slice(tt * 128, (tt + 1) * 128)
                js = slice(j * 512, (j + 1) * 512)
                if j == 0:
                    o_tiles[tt] = opool.tile([128, D], BF16, tag="o",
                                             name=f"o{tt}")
                o_t = o_tiles[tt]
                ob = pool.tile([128, 512], F32,
                               tag=("ps2" if pool is psA else "s"),
                               name=f"ob{tt}_{j}")
                for fi in range(FT):
                    nc.tensor.matmul(
                        ob[:], ctx_sb[:, fi, ts], wo_sb[:, fi, js],
                        start=(fi == 0), stop=(fi == FT - 1),
                    )
                if copy_scalar:
                    nc.scalar.copy(o_t[:, js], ob[:])
                else:
                    nc.vector.tensor_copy(o_t[:, js], ob[:])
                if j == 1:
                    dma_eng.dma_start(out=out[ts, :], in_=o_t[:])

            p_tiles = {}

            def scores_exp(hp, wq_i, cargo=None):
                ws = slice(wq_i * 512, (wq_i + 1) * 512)
                p_pair = (
                    ppool.tile([128, KT // 2, 1024], BF16, tag="p",
                               name=f"p{hp}_{wq_i}a"),
                    ppool.tile([128, KT // 2, 1024], BF16, tag="p",
                               name=f"p{hp}_{wq_i}b"),
                )
                p_tiles[(hp, wq_i)] = p_pair
                for kt in range(KT):
                    ks = slice(kt * 128, (kt + 1) * 128)
                    s_t = psS.tile([128, 1024], F32, tag="s",
                                   name=f"s{hp}{wq_i}_{kt}")
                    nc.tensor.matmul(
                        s_t[:, 0:512], k_sb[0:64, hp, ks],
                        q_sb[0:64, hp, ws], start=True, stop=True,
                    )
                    nc.tensor.matmul(
                        s_t[:, 512:1024], k_sb[64:128, hp, ks],
                        q_sb[64:128, hp, ws], start=True, stop=True,
                    )
                    nc.scalar.activation(
                        p_pair[kt // 8][:, kt % 8, :], s_t[:], AFT.Exp)
                    if cargo is not None:
                        cargo(kt)

            def ctx_mms(hp, wq_i, c_h0, c_h1, kt):
                h0, h1 = 2 * hp, 2 * hp + 1
                p_half = p_tiles[(hp, wq_i)][kt // 8]
                nc.tensor.matmul(
                    c_h0[:], vt_sb[:, h0, kt, :], p_half[:, kt % 8, 0:512],
                    start=(kt == 0), stop=(kt == KT - 1),
                )
                nc.tensor.matmul(
                    c_h1[:], vt_sb[:, h1, kt, :],
                    p_half[:, kt % 8, 512:1024],
                    start=(kt == 0), stop=(kt == KT - 1),
                )

            def ctx_alloc(hp, wq_i, pool=None, tag="c"):
                pool = pool if pool is not None else psC
                c_h0 = pool.tile([65, 512], F32, tag=tag,
                                 name=f"c{hp}{wq_i}a")
                c_h1 = pool.tile([65, 512], F32, tag=tag,
                                 name=f"c{hp}{wq_i}b")
                return c_h0, c_h1

            def ctx_norm(hp, wq_i, mms_done, dma_eng=None):
                dma_eng = dma_eng if dma_eng is not None else nc.sync
                h0, h1 = 2 * hp, 2 * hp + 1
                ws = slice(wq_i * 512, (wq_i + 1) * 512)
                c_h0, c_h1 = mms_done
                p_tiles.pop((hp, wq_i))
                # normalize h0 -> ctx_sb[0:64]; h1 -> tmp + DMA shift.
                # (the l-row must be tensor_copy'd off partition 64 first:
                # a custom-DVE op straight from PSUM@p64 to SBUF@p0
                # returns garbage on HW)
                lrow0 = small.tile([1, 512], F32, tag="lr",
                                   name=f"lr0_{hp}{wq_i}")
                nc.vector.tensor_copy(lrow0[:], c_h0[64:65, :])
                linv0 = small.tile([1, 512], F32, tag="linv",
                                   name=f"l0_{hp}{wq_i}")
                nc.vector.reciprocal_approx_fast(linv0[:], lrow0[:])
                lb0 = small.tile([64, 512], F32, tag="lb",
                                 name=f"lb0_{hp}{wq_i}")
                nc.gpsimd.partition_broadcast(lb0[:], linv0[:])
                nc.vector.tensor_mul(
                    ctx_sb[0:64, hp, ws], c_h0[0:64, :], lb0[:])

                lrow1 = small.tile([1, 512], F32, tag="lr",
                                   name=f"lr1_{hp}{wq_i}")
                nc.vector.tensor_copy(lrow1[:], c_h1[64:65, :])
                linv1 = small.tile([1, 512], F32, tag="linv",
                                   name=f"l1_{hp}{wq_i}")
                nc.vector.reciprocal_approx_fast(linv1[:], lrow1[:])
                lb1 = small.tile([64, 512], F32, tag="lb",
                                 name=f"lb1_{hp}{wq_i}")
                nc.gpsimd.partition_broadcast(lb1[:], linv1[:])
                tmp1 = small.tile([64, 512], BF16, tag="tmp",
                                  name=f"t1_{hp}{wq_i}")
                nc.vector.tensor_mul(tmp1[:], c_h1[0:64, :], lb1[:])
                dma_eng.dma_start(out=ctx_sb[64:128, hp, ws], in_=tmp1[:])

            # cargo closures per window
            def w0_cargo(kt):
                if kt < 2:
                    kproj_quad(1, 2 + kt, kp_tiles[1])
                elif kt < 6:
                    if kt == 2:
                        kp_tiles[2] = kproj_alloc(2)
                    kproj_quad(2, kt - 2, kp_tiles[2])
                elif kt < 10:
                    if kt == 6:
                        kp_tiles[3] = kproj_alloc(3)
                    kproj_quad(3, kt - 6, kp_tiles[3])
                elif kt < 14:
                    vt_kt(kt - 10)

            def w1_cargo(kt):
                if kt < 12:
                    vt_kt(4 + kt)
                else:
                    if kt == 12:
                        qp_tiles[1] = qproj_alloc(1)
                    for j in range(4 * (kt - 12), 4 * (kt - 12) + 4):
                        qproj_mm(1, j, qp_tiles[1])

            # ctx pairs slide to slots 4-15 (doubles early, at 5/7/9/11)
            # so the first pair never waits on the previous job's norm
            # chain (psC bank WAR); the other cargo rides slots 0-3
            # (q-proj) or 12-15 (out-proj, whose DVE copies would collide
            # with the norm chain if run at the window start).
            CTX_SLOTS = {4: (0,), 5: (1, 2), 6: (3,), 7: (4, 5), 8: (6,),
                         9: (7, 8), 10: (9,), 11: (10, 11), 12: (12,),
                         13: (13,), 14: (14,), 15: (15,)}

            def ctx_cargo(hp, wq_i, extra=None, extra_slots=(0, 1, 2, 3)):
                c = ctx_alloc(hp, wq_i)

                def cargo(kt):
                    for ckt in CTX_SLOTS.get(kt, ()):
                        ctx_mms(hp, wq_i, c[0], c[1], ckt)
                    if extra is not None and kt in extra_slots:
                        extra(extra_slots.index(kt))
                return c, cargo

            def qproj_cargo(c):
                """q-proj chunk c as 4-MM quads."""
                def cargo(s):
                    if s == 0:
                        qp_tiles[c] = qproj_alloc(c)
                    for j in range(4 * s, 4 * s + 4):
                        qproj_mm(c, j, qp_tiles[c])
                return cargo

            def ttq_cargo(tt_base):
                """4 half-tts (= 2 full tt)."""
                def cargo(s):
                    half_tt(tt_base + s // 2, s % 2)
                return cargo

            # ---- the 8 braided windows ----
            scores_exp(0, 0, cargo=w0_cargo)                       # w0
            # DMA part 3
            xv_t[2] = xchunk(xkpool, "k", 2, xv)
            xv_t[3] = xchunk(xkpool, "k", 3, xv)
            wo_sb = wpool.tile([128, FT, D], BF16, tag="wo")
            nc.sync.dma_start(out=wo_sb[:], in_=wo[:])
            xq_t[2] = xchunk(xqpool, "q", 2, xq)
            scores_exp(1, 0, cargo=w1_cargo)                       # w1
            xq_t[3] = xchunk(xqpool, "q", 3, xq)

            c00, h00 = ctx_cargo(0, 0, extra=qproj_cargo(2))
            scores_exp(0, 1, cargo=h00)                            # w2
            ctx_norm(0, 0, mms_done=c00)
            c10, h10 = ctx_cargo(1, 0, extra=qproj_cargo(3))
            scores_exp(1, 1, cargo=h10)                            # w3
            ctx_norm(1, 0, mms_done=c10)
            TT_SLOTS = (12, 13, 14, 15)
            c01, h01 = ctx_cargo(0, 1, extra=ttq_cargo(0),
                                 extra_slots=TT_SLOTS)
            scores_exp(0, 2, cargo=h01)                            # w4
            ctx_norm(0, 1, mms_done=c01)
            c11, h11 = ctx_cargo(1, 1, extra=ttq_cargo(2),
                                 extra_slots=TT_SLOTS)
            scores_exp(1, 2, cargo=h11)                            # w5
            ctx_norm(1, 1, mms_done=c11)
            c02, h02 = ctx_cargo(0, 2, extra=ttq_cargo(4),
                                 extra_slots=TT_SLOTS)
            scores_exp(0, 3, cargo=h02)                            # w6
            ctx_norm(0, 2, mms_done=c02)
            c12, h12 = ctx_cargo(1, 2, extra=ttq_cargo(6),
                                 extra_slots=TT_SLOTS)
            scores_exp(1, 3, cargo=h12)                            # w7
            ctx_norm(1, 2, mms_done=c12)

            # ---- dense tail ----
            # ctx(0,3) borrows the scores PSUM banks (scores are done) so
            # it does not wait on norm(1,2)'s reads of the psC banks;
            # ctx(1,3) takes psC once norm(1,2) drains (hidden under
            # ctx(0,3)'s 7us of MMs).  Shift DMAs ride the idle ACT
            # engine's queue; tt out-proj MMs keep the PE hot under the
            # norm chains.
            c03 = ctx_alloc(0, 3, pool=psS, tag="s")
            for kt in range(KT):
                ctx_mms(0, 3, c03[0], c03[1], kt)
            ctx_norm(0, 3, mms_done=c03, dma_eng=nc.scalar)
            c13 = ctx_alloc(1, 3)
            for kt in range(KT):
                ctx_mms(1, 3, c13[0], c13[1], kt)
            half_tt(8, 0, pool=psS, copy_scalar=True)
            half_tt(8, 1)
            half_tt(9, 0, pool=psS, copy_scalar=True)
            half_tt(9, 1)
            ctx_norm(1, 3, mms_done=c13, dma_eng=nc.scalar)
            for tt in (10, 11, 12, 13, 14, 15):
                half_tt(tt, 0, pool=psS, copy_scalar=True)
                half_tt(tt, 1)

    nc.compile()
    return nc


def get_program():
    if "nc" not in _CACHE:
        _CACHE["nc"] = _build()
    return _CACHE["nc"]


def _bf(a):
    import ml_dtypes
    return a.astype(ml_dtypes.bfloat16)


def prep_in_maps(query_tensor, key_tensor, value_tensor, w_q, b_q, w_k, b_k,
                 w_v, b_v, w_out, b_out):
    """Per-core input dicts. Core c: batch c//4, feature rows [256*(c%4), ...)."""
    f32 = np.float32
    scale = f32(1.0 / np.sqrt(DK))

    def xt(x, b):  # [S, D] -> [KC, 128, DT, 512] (chunk-major, contiguous)
        return _bf(np.ascontiguousarray(
            np.asarray(x[b], f32).reshape(KC, 512, DT, 128)
            .transpose(0, 3, 2, 1)))

    xs = {"xq_t": [xt(query_tensor, b) for b in range(B)],
          "xk_t": [xt(key_tensor, b) for b in range(B)],
          "xv_t": [xt(value_tensor, b) for b in range(B)]}

    def wt(w, g, s=f32(1.0)):  # rows [256g, 256g+256) of w -> [128, DT, F]
        sl = np.asarray(w[256 * g:256 * (g + 1), :], f32) * s  # [F, D]
        return _bf(np.ascontiguousarray(
            sl.T.reshape(DT, 128, F).transpose(1, 0, 2)))

    def wvt(w, g):  # [DT, 128, F] (d-major, untransposed)
        sl = np.asarray(w[256 * g:256 * (g + 1), :], f32)  # [F, D]
        return _bf(np.ascontiguousarray(sl.T.reshape(DT, 128, F)))

    def bt(b_, g, s=f32(1.0)):  # [128, FT]
        sl = np.asarray(b_[256 * g:256 * (g + 1)], f32) * s
        return np.ascontiguousarray(sl.reshape(FT, 128).T)

    def wot(w, g):  # cols [256g, 256g+256) of w_out -> [128, FT, D]
        sl = np.asarray(w[:, 256 * g:256 * (g + 1)], f32)  # [D, F]
        return _bf(np.ascontiguousarray(
            sl.T.reshape(FT, 128, D).transpose(1, 0, 2)))

    in_maps = []
    for c in range(N_CORES):
        b, g = divmod(c, GROUPS)
        in_maps.append({
            "xq_t": xs["xq_t"][b], "xk_t": xs["xk_t"][b], "xv_t": xs["xv_t"][b],
            "wq_t": wt(w_q, g, scale), "wk_t": wt(w_k, g),
            "wv_t2": wvt(w_v, g),
            "bq": bt(b_q, g, scale), "bk": bt(b_k, g),
            "bv_row": np.ascontiguousarray(
                np.asarray(b_v[256 * g:256 * (g + 1)], f32).reshape(1, F)),
            "wo_t": wot(w_out, g),
        })
    return in_maps


def kernel(query_tensor, key_tensor, value_tensor, w_q, b_q, w_k, b_k,
           w_v, b_v, w_out, b_out):
    global LAST_RESULTS
    nc = get_program()
    in_maps = prep_in_maps(query_tensor, key_tensor, value_tensor, w_q, b_q,
                           w_k, b_k, w_v, b_v, w_out, b_out)
    res = run_bass_kernel_spmd(nc, in_maps, list(range(N_CORES)),
                               tmpdir=os.environ.get("BASS_TMPDIR"))
    LAST_RESULTS = res
    b_out = np.asarray(b_out, np.float32)
    out = np.empty((B, S, D), np.float32)
    for b in range(B):
        acc = res.results[4 * b]["out_p"].astype(np.float32)
        for g in range(1, GROUPS):
            acc = acc + res.results[4 * b + g]["out_p"]
        out[b] = acc + b_out
    return out


# revision 42
# speedup vs baseline: 1.0576x; 1.0064x over previous
"""Multi-head attention (B=2, S=2048, D=1024, H=16) on 8 TRN2 NeuronCores.

Sharding (Megatron-style, hardcoded):
  - batch b = core // 4  (2 groups of 4 cores)
  - head group g = core % 4 -> heads [4g, 4g+4), feature slice F = 256 rows
    of w_q/w_k/w_v (column-parallel) and 256 columns of w_out (row-parallel).
Each core computes a full [S, D] partial of the output for its batch
(summed over its 256 ctx features); the host sums the 4 partials per batch
and adds b_out.

On-core design (v7) — the ACT engine (exp) is the pacer: 128 exps of
[128,1024] x ~1.11us = 142us is the floor, so everything else is cargo
braided into the 8 ACT-paced score windows:
  - x tensors live in DRAM as [128, DT, S] so one 512-column chunk is a
    single DMA descriptor ([128, DT, 512]); the SP sequencer costs ~600ns
    per descriptor and was the v6 head bottleneck.
  - head (~22us): short warmup, k-proj c0 + q-proj c0 + half of k-proj c1
    as soon as their DMAs land; first exp fires ~22us (v5: 50us).
  - w0 = scores(0,0) + k-proj c1(rest),c2,c3 just ahead of their consuming
    kt + vt 0-3;  w1 = scores(1,0) + vt 4-15 + q-proj c1.
  - w2..w7 = scores(job) + ctx(job-2) + cargo (q2, q3, out-proj tt0-7).
  - tail = ctx(0,3); norm(0,3); ctx(1,3) (PSUM banks borrowed from the
    now-idle scores pool) overlapping norm(0,3) DVE chain; tt8-11
    overlapping norm(1,3); tt12-15. MMs stay back-to-back for HAM.
  - per-job p is stored as two [128,8,1024] halves (bufs=6) so the
    rotation frees at half-window granularity (v6 stalled ~1us per
    window start on whole-p WAR).
Per-op idioms unchanged from v5 (all HW-verified): paired K=64 score
matmuls into one [128,1024] PSUM tile -> single exp for 2 heads x 512
queries; v^T by x-stationary projection; ones-column softmax denominator
(M=65 ctx stationary); h' ctx normalized into a bp-0 temp and DMA-shifted
to partitions 64-127 (DVE lanes cannot cross partitions; the l-row is
tensor_copy'd off partition 64 before the custom-DVE reciprocal).
Softmax skips the max-subtraction: scores ~ N(0,1), exp never overflows.
"""

import os

import numpy as np

import concourse.bass as bass
import concourse.tile as tile
from concourse import bacc, mybir
from concourse.bass_utils import run_bass_kernel_spmd

B, S, D, H, DK = 2, 2048, 1024, 16, 64
N_CORES = 8
GROUPS = 4              # head-groups (cores per batch)
HL = H // GROUPS        # heads per core = 4
F = HL * DK             # feature slice per core = 256
FT = F // 128           # f-tiles per core = 2
DT = D // 128           # d-tiles (contraction) = 8
KT = S // 128           # 128-wide key tiles = 16
WQ = S // 512           # 512-wide query chunks = 4
KC = S // 512           # 512-wide key chunks for k-proj = 4

F32 = mybir.dt.float32
BF16 = mybir.dt.bfloat16
AFT = mybir.ActivationFunctionType

_CACHE = {}
LAST_RESULTS = None  # BassKernelResults of the most recent run (for test.py)


def _build():
    nc = bacc.Bacc("TRN2", target_bir_lowering=False, debug=False,
                   num_devices=N_CORES)

    xq = nc.declare_dram_parameter("xq_t", [KC, 128, DT, 512], BF16,
                                   isOutput=False)
    xk = nc.declare_dram_parameter("xk_t", [KC, 128, DT, 512], BF16,
                                   isOutput=False)
    xv = nc.declare_dram_parameter("xv_t", [KC, 128, DT, 512], BF16,
                                   isOutput=False)
    wq = nc.declare_dram_parameter("wq_t", [128, DT, F], BF16, isOutput=False)
    wk = nc.declare_dram_parameter("wk_t", [128, DT, F], BF16, isOutput=False)
    wv = nc.declare_dram_parameter("wv_t2", [DT, 128, F], BF16, isOutput=False)
    bq = nc.declare_dram_parameter("bq", [128, FT], F32, isOutput=False)
    bk = nc.declare_dram_parameter("bk", [128, FT], F32, isOutput=False)
    bv = nc.declare_dram_parameter("bv_row", [1, F], F32, isOutput=False)
    wo = nc.declare_dram_parameter("wo_t", [128, FT, D], BF16, isOutput=False)
    out = nc.declare_dram_parameter("out_p", [S, D], BF16, isOutput=True)

    with tile.TileContext(nc) as tc:
        with (
            tc.tile_pool(name="const", bufs=1) as const,
            tc.tile_pool(name="acts", bufs=1) as acts,
            tc.tile_pool(name="ppool", bufs=6) as ppool,
            tc.tile_pool(name="xkpool", bufs=1) as xkpool,
            tc.tile_pool(name="xqpool", bufs=1) as xqpool,
            tc.tile_pool(name="wpool", bufs=1) as wpool,
            tc.tile_pool(name="small", bufs=1) as small,
            tc.tile_pool(name="opool", bufs=2) as opool,
            tc.tile_pool(name="psS", bufs=2, space="PSUM") as psS,
            tc.tile_pool(name="psC", bufs=2, space="PSUM") as psC,
            tc.tile_pool(name="psA", bufs=2, space="PSUM") as psA,
        ):
            scratch = const.tile([128, 512], BF16, tag="scratch")
            nc.gpsimd.memset(scratch[:], 0.0)

            # persistent activations
            q_sb = acts.tile([128, FT, S], BF16, tag="q")
            k_sb = acts.tile([128, FT, S], BF16, tag="k")
            vt_sb = acts.tile([128, HL, KT, 65], BF16, tag="vt")
            ctx_sb = acts.tile([128, FT, S], BF16, tag="ctx")
            nc.vector.memset(vt_sb[:, :, :, 64:65], 1.0)

            def xchunk(pool, pfx, c, src=None, eng=None):
                t = pool.tile([128, DT, 512], BF16, tag="x" + pfx,
                              bufs=(2 if pfx == "q" else KC),
                              name=f"x{pfx}{c}")
                if src is not None:
                    (eng or nc.sync).dma_start(out=t[:], in_=src[c])
                return t

            # ---- DMA part 1: what the head needs (the early aggregate
            # DMA rate ~225 B/ns is the physical bound; fewer, bigger
            # descriptors win because SP descriptor-gen is 600ns each) ----
            w_sb = {}
            xk_t = [None] * KC
            xq_t = [None] * WQ
            w_sb["k"] = wpool.tile([128, DT, F], BF16, tag="wk", name="wk_sb")
            nc.sync.dma_start(out=w_sb["k"][:], in_=wk[:])
            xk_t[0] = xchunk(xkpool, "k", 0, xk)
            w_sb["q"] = wpool.tile([128, DT, F], BF16, tag="wq", name="wq_sb")
            nc.sync.dma_start(out=w_sb["q"][:], in_=wq[:])
            xq_t[0] = xchunk(xqpool, "q", 0, xq)
            b_sb = {}
            for name, bp in (("k", bk), ("q", bq)):
                b_sb[name] = const.tile([128, FT], F32, tag=f"b{name}",
                                        name=f"b{name}_sb")
                nc.sync.dma_start(out=b_sb[name][:], in_=bp[:])
            bv_row = const.tile([1, F], F32, tag="bvrow")
            nc.sync.dma_start(out=bv_row[:], in_=bv[:])
            bv_b = const.tile([128, F], F32, tag="bvb")
            nc.gpsimd.partition_broadcast(bv_b[:], bv_row[:])

            # ---- head compute: warmup + k-proj c0 + q-proj c0 + kc1 ----
            warm = psA.tile([128, 512], F32, tag="ps2", name="warm")
            for i in range(20):
                nc.tensor.matmul(warm[:], scratch[:, 0:128], scratch[:],
                                 start=True, stop=True)

            def kproj_quad(c, qd, tiles):
                """Emit dt = 2*qd, 2*qd+1 (x fi 0,1) of k-proj chunk c into
                tiles [fi0, fi1]; at qd==3 add bias into k_sb."""
                ks = slice(c * 512, (c + 1) * 512)
                for dt in (2 * qd, 2 * qd + 1):
                    for fi in range(FT):
                        nc.tensor.matmul(
                            tiles[fi][:],
                            w_sb["k"][:, dt, fi * 128:(fi + 1) * 128],
                            xk_t[c][:, dt, :],
                            start=(dt == 0), stop=(dt == DT - 1),
                        )
                if qd == 3:
                    for fi in range(FT):
                        nc.vector.tensor_scalar_add(
                            out=k_sb[:, fi, ks], in0=tiles[fi][:],
                            scalar1=b_sb["k"][:, fi:fi + 1],
                        )

            def kproj_alloc(c):
                return [psA.tile([128, 512], F32, tag="ps2",
                                 name=f"kb{c}_{fi}") for fi in range(FT)]

            def qproj_mm(c, j, tiles):
                """Emit MM j (0..15) of q-proj chunk c: dt=j//2, fi=j%2."""
                ws = slice(c * 512, (c + 1) * 512)
                dt, fi = j // 2, j % 2
                nc.tensor.matmul(
                    tiles[fi][:],
                    w_sb["q"][:, dt, fi * 128:(fi + 1) * 128],
                    xq_t[c][:, dt, :], start=(dt == 0), stop=(dt == DT - 1),
                )
                if j == 15:
                    for fi in range(FT):
                        nc.vector.tensor_scalar_add(
                            out=q_sb[:, fi, ws], in0=tiles[fi][:],
                            scalar1=b_sb["q"][:, fi:fi + 1],
                        )

            def qproj_alloc(c):
                return [psA.tile([128, 512], F32, tag="ps2",
                                 name=f"qb{c}_{fi}") for fi in range(FT)]

            kp_tiles, qp_tiles = {}, {}

            kp_tiles[0] = kproj_alloc(0)
            for qd in range(4):
                kproj_quad(0, qd, kp_tiles[0])
            qp_tiles[0] = qproj_alloc(0)
            for j in range(16):
                qproj_mm(0, j, qp_tiles[0])
            # ---- DMA part 2 ----
            for c in range(1, KC):
                xk_t[c] = xchunk(xkpool, "k", c, xk)
            wv_sb = wpool.tile([128, DT, F], BF16, tag="wv")
            for dt in range(DT):
                nc.sync.dma_start(out=wv_sb[:, dt, :], in_=wv[dt])
            # xv chunk tiles reuse the xk slots (kproj chunk c's reads are
            # long done before xv chunk c lands)
            xv_t = [None] * KC
            xv_t[0] = xchunk(xkpool, "k", 0, xv)
            xq_t[1] = xchunk(xqpool, "q", 1, xq)
            xv_t[1] = xchunk(xkpool, "k", 1, xv)
            # head gets a 3-quad head start on k-proj c1
            kp_tiles[1] = kproj_alloc(1)
            kproj_quad(1, 0, kp_tiles[1])
            kproj_quad(1, 1, kp_tiles[1])
            kproj_quad(1, 2, kp_tiles[1])

            # ---- braid machinery ----
            def vt_kt(kt):
                vb = psA.tile([128, 512], F32, tag="ps2", name=f"vtb{kt}")
                cc, ko = kt // 4, (kt % 4) * 128
                for dt in range(DT):
                    nc.tensor.matmul(
                        vb[:, 0:F], xv_t[cc][:, dt, ko:ko + 128],
                        wv_sb[:, dt, :],
                        start=(dt == 0), stop=(dt == DT - 1),
                    )
                nc.vector.tensor_add(
                    vt_sb[:, :, kt, 0:64], vb[:, 0:F], bv_b[:])

            o_tiles = {}

            def half_tt(tt, j, pool=None, dma_eng=None, copy_scalar=False):
                """Half out-proj for row-tile tt: out cols [512j, 512j+512)."""
                pool = pool if pool is not None else psA
                dma_eng = dma_eng if dma_eng is not None else nc.sync
                ts = slice(tt * 128, (tt + 1) * 128)
                js = slice(j * 512, (j + 1) * 512)
                if j == 0:
                    o_tiles[tt] = opool.tile([128, D], BF16, tag="o",
                                             name=f"o{tt}")
                o_t = o_tiles[tt]
                ob = pool.tile([128, 512], F32,
                               tag=("ps2" if pool is psA else "s"),
                               name=f"ob{tt}_{j}")
                for fi in range(FT):
                    nc.tensor.matmul(
                        ob[:], ctx_sb[:, fi, ts], wo_sb[:, fi, js],
                        start=(fi == 0), stop=(fi == FT - 1),
                    )
                if copy_scalar:
                    nc.scalar.copy(o_t[:, js], ob[:])
                else:
                    nc.vector.tensor_copy(o_t[:, js], ob[:])
                if j == 1:
                    dma_eng.dma_start(out=out[ts, :], in_=o_t[:])

            p_tiles = {}

            def scores_exp(hp, wq_i, cargo=None):
                ws = slice(wq_i * 512, (wq_i + 1) * 512)
                p_pair = (
                    ppool.tile([128, KT // 2, 1024], BF16, tag="p",
                               name=f"p{hp}_{wq_i}a"),
                    ppool.tile([128, KT // 2, 1024], BF16, tag="p",
                               name=f"p{hp}_{wq_i}b"),
                )
                p_tiles[(hp, wq_i)] = p_pair
                for kt in range(KT):
                    ks = slice(kt * 128, (kt + 1) * 128)
                    s_t = psS.tile([128, 1024], F32, tag="s",
                                   name=f"s{hp}{wq_i}_{kt}")
                    nc.tensor.matmul(
                        s_t[:, 0:512], k_sb[0:64, hp, ks],
                        q_sb[0:64, hp, ws], start=True, stop=True,
                    )
                    nc.tensor.matmul(
                        s_t[:, 512:1024], k_sb[64:128, hp, ks],
                        q_sb[64:128, hp, ws], start=True, stop=True,
                    )
                    nc.scalar.activation(
                        p_pair[kt // 8][:, kt % 8, :], s_t[:], AFT.Exp)
                    if cargo is not None:
                        cargo(kt)

            def ctx_mms(hp, wq_i, c_h0, c_h1, kt):
                h0, h1 = 2 * hp, 2 * hp + 1
                p_half = p_tiles[(hp, wq_i)][kt // 8]
                nc.tensor.matmul(
                    c_h0[:], vt_sb[:, h0, kt, :], p_half[:, kt % 8, 0:512],
                    start=(kt == 0), stop=(kt == KT - 1),
                )
                nc.tensor.matmul(
                    c_h1[:], vt_sb[:, h1, kt, :],
                    p_half[:, kt % 8, 512:1024],
                    start=(kt == 0), stop=(kt == KT - 1),
                )

            def ctx_alloc(hp, wq_i, pool=None, tag="c"):
                pool = pool if pool is not None else psC
                c_h0 = pool.tile([65, 512], F32, tag=tag,
                                 name=f"c{hp}{wq_i}a")
                c_h1 = pool.tile([65, 512], F32, tag=tag,
                                 name=f"c{hp}{wq_i}b")
                return c_h0, c_h1

            def ctx_norm(hp, wq_i, mms_done, dma_eng=None):
                dma_eng = dma_eng if dma_eng is not None else nc.sync
                h0, h1 = 2 * hp, 2 * hp + 1
                ws = slice(wq_i * 512, (wq_i + 1) * 512)
                c_h0, c_h1 = mms_done
                p_tiles.pop((hp, wq_i))
                # normalize h0 -> ctx_sb[0:64]; h1 -> tmp + DMA shift.
                # (the l-row must be tensor_copy'd off partition 64 first:
                # a custom-DVE op straight from PSUM@p64 to SBUF@p0
                # returns garbage on HW)
                lrow0 = small.tile([1, 512], F32, tag="lr",
                                   name=f"lr0_{hp}{wq_i}")
                nc.vector.tensor_copy(lrow0[:], c_h0[64:65, :])
                linv0 = small.tile([1, 512], F32, tag="linv",
                                   name=f"l0_{hp}{wq_i}")
                nc.vector.reciprocal_approx_fast(linv0[:], lrow0[:])
                lb0 = small.tile([64, 512], F32, tag="lb",
                                 name=f"lb0_{hp}{wq_i}")
                nc.gpsimd.partition_broadcast(lb0[:], linv0[:])
                nc.vector.tensor_mul(
                    ctx_sb[0:64, hp, ws], c_h0[0:64, :], lb0[:])

                lrow1 = small.tile([1, 512], F32, tag="lr",
                                   name=f"lr1_{hp}{wq_i}")
                nc.vector.tensor_copy(lrow1[:], c_h1[64:65, :])
                linv1 = small.tile([1, 512], F32, tag="linv",
                                   name=f"l1_{hp}{wq_i}")
                nc.vector.reciprocal_approx_fast(linv1[:], lrow1[:])
                lb1 = small.tile([64, 512], F32, tag="lb",
                                 name=f"lb1_{hp}{wq_i}")
                nc.gpsimd.partition_broadcast(lb1[:], linv1[:])
                tmp1 = small.tile([64, 512], BF16, tag="tmp",
                                  name=f"t1_{hp}{wq_i}")
                nc.vector.tensor_mul(tmp1[:], c_h1[0:64, :], lb1[:])
                dma_eng.dma_start(out=ctx_sb[64:128, hp, ws], in_=tmp1[:])

            # cargo closures per window
            def w0_cargo(kt):
                if kt < 1:
                    kproj_quad(1, 3, kp_tiles[1])
                elif kt < 5:
                    if kt == 1:
                        kp_tiles[2] = kproj_alloc(2)
                    kproj_quad(2, kt - 1, kp_tiles[2])
                elif kt < 9:
                    if kt == 5:
                        kp_tiles[3] = kproj_alloc(3)
                    kproj_quad(3, kt - 5, kp_tiles[3])
                elif 10 <= kt < 14:
                    vt_kt(kt - 10)

            def w1_cargo(kt):
                if kt < 8:
                    vt_kt(4 + kt)
                elif kt < 12:
                    if kt == 8:
                        qp_tiles[1] = qproj_alloc(1)
                    for j in range(4 * (kt - 8), 4 * (kt - 8) + 4):
                        qproj_mm(1, j, qp_tiles[1])
                else:
                    vt_kt(kt)

            # ctx pairs slide to slots 4-15 (doubles early, at 5/7/9/11)
            # so the first pair never waits on the previous job's norm
            # chain (psC bank WAR); the other cargo rides slots 0-3
            # (q-proj) or 12-15 (out-proj, whose DVE copies would collide
            # with the norm chain if run at the window start).
            CTX_SLOTS = {4: (0,), 5: (1, 2), 6: (3,), 7: (4, 5), 8: (6,),
                         9: (7, 8), 10: (9,), 11: (10, 11), 12: (12,),
                         13: (13,), 14: (14,), 15: (15,)}

            def ctx_cargo(hp, wq_i, extra=None, extra_slots=(0, 1, 2, 3)):
                c = ctx_alloc(hp, wq_i)

                def cargo(kt):
                    for ckt in CTX_SLOTS.get(kt, ()):
                        ctx_mms(hp, wq_i, c[0], c[1], ckt)
                    if extra is not None and kt in extra_slots:
                        extra(extra_slots.index(kt))
                return c, cargo

            def qproj_cargo(c):
                """q-proj chunk c as 4-MM quads."""
                def cargo(s):
                    if s == 0:
                        qp_tiles[c] = qproj_alloc(c)
                    for j in range(4 * s, 4 * s + 4):
                        qproj_mm(c, j, qp_tiles[c])
                return cargo

            def ttq_cargo(tt_base):
                """4 half-tts (= 2 full tt)."""
                def cargo(s):
                    half_tt(tt_base + s // 2, s % 2)
                return cargo

            # ---- the 8 braided windows ----
            scores_exp(0, 0, cargo=w0_cargo)                       # w0
            # DMA part 3
            xv_t[2] = xchunk(xkpool, "k", 2, xv)
            xv_t[3] = xchunk(xkpool, "k", 3, xv)
            wo_sb = wpool.tile([128, FT, D], BF16, tag="wo")
            nc.sync.dma_start(out=wo_sb[:], in_=wo[:])
            xq_t[2] = xchunk(xqpool, "q", 2, xq)
            scores_exp(1, 0, cargo=w1_cargo)                       # w1
            xq_t[3] = xchunk(xqpool, "q", 3, xq)

            c00, h00 = ctx_cargo(0, 0, extra=qproj_cargo(2))
            scores_exp(0, 1, cargo=h00)                            # w2
            ctx_norm(0, 0, mms_done=c00)
            c10, h10 = ctx_cargo(1, 0, extra=qproj_cargo(3))
            scores_exp(1, 1, cargo=h10)                            # w3
            ctx_norm(1, 0, mms_done=c10)
            TT_SLOTS = (12, 13, 14, 15)
            c01, h01 = ctx_cargo(0, 1, extra=ttq_cargo(0),
                                 extra_slots=TT_SLOTS)
            scores_exp(0, 2, cargo=h01)                            # w4
            ctx_norm(0, 1, mms_done=c01)
            c11, h11 = ctx_cargo(1, 1, extra=ttq_cargo(2),
                                 extra_slots=TT_SLOTS)
            scores_exp(1, 2, cargo=h11)                            # w5
            ctx_norm(1, 1, mms_done=c11)
            c02, h02 = ctx_cargo(0, 2, extra=ttq_cargo(4),
                                 extra_slots=TT_SLOTS)
            scores_exp(0, 3, cargo=h02)                            # w6
            ctx_norm(0, 2, mms_done=c02)
            c12, h12 = ctx_cargo(1, 2, extra=ttq_cargo(6),
                                 extra_slots=TT_SLOTS)
            scores_exp(1, 3, cargo=h12)                            # w7
            ctx_norm(1, 2, mms_done=c12)

            # ---- dense tail ----
            # ctx(0,3) borrows the scores PSUM banks (scores are done) so
            # it does not wait on norm(1,2)'s reads of the psC banks;
            # ctx(1,3) takes psC once norm(1,2) drains (hidden under
            # ctx(0,3)'s 7us of MMs).  Shift DMAs ride the idle ACT
            # engine's queue; tt out-proj MMs keep the PE hot under the
            # norm chains.
            c03 = ctx_alloc(0, 3, pool=psS, tag="s")
            for kt in range(KT):
                ctx_mms(0, 3, c03[0], c03[1], kt)
            ctx_norm(0, 3, mms_done=c03, dma_eng=nc.scalar)
            c13 = ctx_alloc(1, 3)
            for kt in range(KT):
                ctx_mms(1, 3, c13[0], c13[1], kt)
            half_tt(8, 0, pool=psS, copy_scalar=True)
            half_tt(8, 1)
            half_tt(9, 0, pool=psS, copy_scalar=True)
            half_tt(9, 1)
            ctx_norm(1, 3, mms_done=c13, dma_eng=nc.scalar)
            for tt in (10, 11, 12, 13, 14, 15):
                half_tt(tt, 0, pool=psS, copy_scalar=True)
                half_tt(tt, 1)

    nc.compile()
    return nc


def get_program():
    if "nc" not in _CACHE:
        _CACHE["nc"] = _build()
    return _CACHE["nc"]


def _bf(a):
    import ml_dtypes
    return a.astype(ml_dtypes.bfloat16)


def prep_in_maps(query_tensor, key_tensor, value_tensor, w_q, b_q, w_k, b_k,
                 w_v, b_v, w_out, b_out):
    """Per-core input dicts. Core c: batch c//4, feature rows [256*(c%4), ...)."""
    f32 = np.float32
    scale = f32(1.0 / np.sqrt(DK))

    def xt(x, b):  # [S, D] -> [KC, 128, DT, 512] (chunk-major, contiguous)
        return _bf(np.ascontiguousarray(
            np.asarray(x[b], f32).reshape(KC, 512, DT, 128)
            .transpose(0, 3, 2, 1)))

    xs = {"xq_t": [xt(query_tensor, b) for b in range(B)],
          "xk_t": [xt(key_tensor, b) for b in range(B)],
          "xv_t": [xt(value_tensor, b) for b in range(B)]}

    def wt(w, g, s=f32(1.0)):  # rows [256g, 256g+256) of w -> [128, DT, F]
        sl = np.asarray(w[256 * g:256 * (g + 1), :], f32) * s  # [F, D]
        return _bf(np.ascontiguousarray(
            sl.T.reshape(DT, 128, F).transpose(1, 0, 2)))

    def wvt(w, g):  # [DT, 128, F] (d-major, untransposed)
        sl = np.asarray(w[256 * g:256 * (g + 1), :], f32)  # [F, D]
        return _bf(np.ascontiguousarray(sl.T.reshape(DT, 128, F)))

    def bt(b_, g, s=f32(1.0)):  # [128, FT]
        sl = np.asarray(b_[256 * g:256 * (g + 1)], f32) * s
        return np.ascontiguousarray(sl.reshape(FT, 128).T)

    def wot(w, g):  # cols [256g, 256g+256) of w_out -> [128, FT, D]
        sl = np.asarray(w[:, 256 * g:256 * (g + 1)], f32)  # [D, F]
        return _bf(np.ascontiguousarray(
            sl.T.reshape(FT, 128, D).transpose(1, 0, 2)))

    in_maps = []
    for c in range(N_CORES):
        b, g = divmod(c, GROUPS)
        in_maps.append({
            "xq_t": xs["xq_t"][b], "xk_t": xs["xk_t"][b], "xv_t": xs["xv_t"][b],
            "wq_t": wt(w_q, g, scale), "wk_t": wt(w_k, g),
            "wv_t2": wvt(w_v, g),
            "bq": bt(b_q, g, scale), "bk": bt(b_k, g),
            "bv_row": np.ascontiguousarray(
                np.asarray(b_v[256 * g:256 * (g + 1)], f32).reshape(1, F)),
            "wo_t": wot(w_out, g),
        })
    return in_maps


def kernel(query_tensor, key_tensor, value_tensor, w_q, b_q, w_k, b_k,
           w_v, b_v, w_out, b_out):
    global LAST_RESULTS
    nc = get_program()
    in_maps = prep_in_maps(query_tensor, key_tensor, value_tensor, w_q, b_q,
                           w_k, b_k, w_v, b_v, w_out, b_out)
    res = run_bass_kernel_spmd(nc, in_maps, list(range(N_CORES)),
                               tmpdir=os.environ.get("BASS_TMPDIR"))
    LAST_RESULTS = res
    b_out = np.asarray(b_out, np.float32)
    out = np.empty((B, S, D), np.float32)
    for b in range(B):
        acc = res.results[4 * b]["out_p"].astype(np.float32)
        for g in range(1, GROUPS):
            acc = acc + res.results[4 * b + g]["out_p"]
        out[b] = acc + b_out
    return out
